# revision 27
# baseline (speedup 1.0000x reference)
"""Trainium2 Bass kernel for nn_CrossAttentionScaler.

Strategy (8 NeuronCores):
  - Attention is sharded over the key axis N_res (R): core i owns R rows
    [512*i, 512*(i+1)) for ALL 8 heads.  Softmax uses multiplicative
    masking: softmax(l + BIG*(m-1)) == m*exp(l)/sum(m*exp(l)) exactly in
    f32 (exp(-1e9) underflows to 0), and logits are tiny (std ~0.15) so no
    max-subtraction is needed.  Per-core partial AV + denominators (ones
    column appended to V) are ReduceScattered over the query axis T, giving
    core i rows [64*i, 64*(i+1)).
  - Post-RS work (recip, gating, output proj, resampled transition) runs on
    each core's 64-row T-shard.  The original-side transition is sequence
    parallel over the same R shard.
  - All LayerNorm affine params are folded into downstream weight matrices
    on the host; on-device LN is pure (x-mu)*rsqrt(var+eps).
  - Matmul operands are bf16 (PSUM accumulates f32); LN stats + residual
    adds are f32.  q/k head dims are zero-padded 48->64 so per-head slices
    never span a 128-partition boundary.
"""

import os
import numpy as np
import ml_dtypes

BF16 = ml_dtypes.bfloat16

# problem dims (hardcoded per contract)
NCORE = 8
T = 512          # N_tok
R = 4096         # N_res
C = 384          # channels
NH = 8           # heads
KD = 48          # head dim
KDP = 64         # padded head dim
HCP = NH * KDP   # 512
F = 4 * C        # 1536
RS = R // NCORE  # 512  per-core key shard
TS = T // NCORE  # 64   per-core query shard
EPS = 1e-5
P = 128

_CACHE = {}
LAST = {}


def _build_program(biases):
    import concourse.bass as bass
    import concourse.bacc as bacc
    import concourse.tile as tile
    import concourse.mybir as mybir
    from concourse.masks import make_identity

    dt = mybir.dt
    AF = mybir.ActivationFunctionType
    ts = bass.ts

    nc = bacc.Bacc("TRN2", target_bir_lowering=False, debug=False,
                   num_devices=NCORE)

    # ---- dram I/O ----
    d_x = nc.dram_tensor("x", [RS, C], dt.float32, kind="ExternalInput")
    d_res = nc.dram_tensor("res", [T, C], dt.float32, kind="ExternalInput")
    d_res64 = nc.dram_tensor("res64", [TS, C], dt.float32, kind="ExternalInput")
    d_maskT = nc.dram_tensor("maskT", [NH, RS, T], dt.bfloat16, kind="ExternalInput")
    d_wq = nc.dram_tensor("wq", [C, HCP], dt.bfloat16, kind="ExternalInput")
    d_wk = nc.dram_tensor("wk", [C, HCP], dt.bfloat16, kind="ExternalInput")
    d_wv = nc.dram_tensor("wv", [C, C], dt.bfloat16, kind="ExternalInput")
    d_wg = nc.dram_tensor("wg", [C, HCP], dt.bfloat16, kind="ExternalInput")
    d_wo = nc.dram_tensor("wo", [HCP, C], dt.bfloat16, kind="ExternalInput")
    d_w1ot = nc.dram_tensor("w1ot", [C, F], dt.bfloat16, kind="ExternalInput")
    d_w2ot = nc.dram_tensor("w2ot", [F, C], dt.bfloat16, kind="ExternalInput")
    d_w1rt = nc.dram_tensor("w1rt", [C, F], dt.bfloat16, kind="ExternalInput")
    d_w2rt = nc.dram_tensor("w2rt", [F, C], dt.bfloat16, kind="ExternalInput")
    d_gb = nc.dram_tensor("gb", [1, HCP], dt.bfloat16, kind="ExternalInput")
    d_bias = {}
    for name, shp in [("qb", [1, HCP]), ("kb", [1, HCP]), ("vb", [1, C]),
                      ("ob", [1, C]), ("b1ot", [1, F]), ("b2ot", [1, C]),
                      ("b1rt", [1, F]), ("b2rt", [1, C])]:
        if biases[name]:
            d_bias[name] = nc.dram_tensor(name, shp, dt.bfloat16,
                                          kind="ExternalInput")
    d_orig_out = nc.dram_tensor("orig_out", [RS, C], dt.float32,
                                kind="ExternalOutput")
    d_res_out = nc.dram_tensor("res_out", [TS, C], dt.float32,
                               kind="ExternalOutput")

    with tile.TileContext(nc) as tc:
        import contextlib
        with contextlib.ExitStack() as ctx:
            const = ctx.enter_context(tc.tile_pool(name="const", bufs=1))
            pers = ctx.enter_context(tc.tile_pool(name="pers", bufs=1))
            stats = ctx.enter_context(tc.tile_pool(name="stats", bufs=8))
            maskp = ctx.enter_context(tc.tile_pool(name="maskp", bufs=3))
            ep = ctx.enter_context(tc.tile_pool(name="ep", bufs=4))
            emp = ctx.enter_context(tc.tile_pool(name="emp", bufs=3))
            origp = ctx.enter_context(tc.tile_pool(name="origp", bufs=2))
            psp = ctx.enter_context(tc.tile_pool(name="psp", bufs=4, space="PSUM"))
            avp = ctx.enter_context(tc.tile_pool(name="avp", bufs=4, space="PSUM"))
            dram = ctx.enter_context(tc.tile_pool(name="dram", bufs=1, space="DRAM"))

            # ---- constants ----
            ident = const.tile([P, P], dt.bfloat16)
            make_identity(nc, ident[:])
            ones = const.tile([1, T], dt.bfloat16)
            nc.vector.memset(ones[:], 1.0)
            epst = const.tile([P, 1], dt.float32)
            nc.vector.memset(epst[:], EPS)
            zerot = const.tile([P, 1], dt.float32)
            nc.vector.memset(zerot[:], 0.0)

            def load_w(dram_t, k, n, name):
                t = const.tile([P, k, n], dt.bfloat16, tag=name)
                nc.sync.dma_start(
                    out=t[:], in_=dram_t.ap().rearrange("(k p) n -> p k n", p=P))
                return t

            wq = load_w(d_wq, 3, HCP, "wq")
            wk = load_w(d_wk, 3, HCP, "wk")
            wv = load_w(d_wv, 3, C, "wv")
            wg = load_w(d_wg, 3, HCP, "wg")
            wo = load_w(d_wo, 4, C, "wo")
            w1ot = load_w(d_w1ot, 3, F, "w1ot")
            w2ot = load_w(d_w2ot, 12, C, "w2ot")
            w1rt = load_w(d_w1rt, 3, F, "w1rt")
            w2rt = load_w(d_w2rt, 12, C, "w2rt")
            gb = const.tile([1, HCP], dt.bfloat16, tag="gb")
            nc.sync.dma_start(out=gb[:], in_=d_gb.ap())
            sb_bias = {}
            for name, hd in d_bias.items():
                shp = [1, hd.shape[1]]
                sb_bias[name] = const.tile(shp, dt.bfloat16, tag="b_" + name)
                nc.sync.dma_start(out=sb_bias[name][:], in_=hd.ap())

            # ---- load activations ----
            x_sb = pers.tile([P, 4, C], dt.float32, tag="x_sb")
            res_sb = pers.tile([P, 4, C], dt.float32, tag="res_sb")
            for m in range(4):
                nc.sync.dma_start(
                    out=x_sb[:, m, :],
                    in_=d_x.ap().rearrange("(m p) c -> p m c", p=P)[:, m, :])
                nc.sync.dma_start(
                    out=res_sb[:, m, :],
                    in_=d_res.ap().rearrange("(m p) c -> p m c", p=P)[:, m, :])
            res64 = pers.tile([TS, C], dt.float32, tag="res64")
            nc.sync.dma_start(out=res64[:], in_=d_res64.ap())

            # ---- layernorm (pure) ----
            def ln(dst, src, np_, tagp):
                # src/dst: [np_, C] APs
                st = stats.tile([P, 6], dt.float32, tag="st" + tagp)
                mv = stats.tile([P, 2], dt.float32, tag="mv" + tagp)
                sd = stats.tile([P, 1], dt.float32, tag="sd" + tagp)
                nc.vector.bn_stats(out=st[:np_], in_=src)
                nc.vector.bn_aggr(out=mv[:np_], in_=st[:np_])
                nc.scalar.activation(out=sd[:np_], in_=mv[:np_, 1:2],
                                     func=AF.Sqrt, bias=epst[:np_], scale=1.0)
                nc.vector.reciprocal(out=sd[:np_], in_=sd[:np_])
                nc.vector.tensor_scalar(out=dst, in0=src,
                                        scalar1=mv[:np_, 0:1], scalar2=sd[:np_],
                                        op0=mybir.AluOpType.subtract,
                                        op1=mybir.AluOpType.mult)

            xh = pers.tile([P, 4, C], dt.bfloat16, tag="xh")
            rh = pers.tile([P, 4, C], dt.bfloat16, tag="rh")
            for m in range(4):
                ln(xh[:, m, :], x_sb[:, m, :], P, "x%d" % m)
                ln(rh[:, m, :], res_sb[:, m, :], P, "r%d" % m)
            rh64 = pers.tile([TS, C], dt.bfloat16, tag="rh64")
            ln(rh64[:], res64[:], TS, "r64")

            # ---- transposes: xhT/rhT [128, 3, 512] via DMA transpose ----
            xhT = pers.tile([P, 3, RS], dt.bfloat16, tag="xhT")
            rhT = pers.tile([P, 3, T], dt.bfloat16, tag="rhT")
            for m in range(4):
                for k in range(3):
                    nc.sync.dma_start_transpose(
                        out=xhT[:, k, ts(m, P)], in_=xh[:, m, ts(k, P)])
                    nc.sync.dma_start_transpose(
                        out=rhT[:, k, ts(m, P)], in_=rh[:, m, ts(k, P)])

            def pe_transpose(dst, src, np_):
                # src [np_, 3*128] -> dst [128, 3, np_]
                for j in range(3):
                    tp = psp.tile([P, np_], dt.bfloat16, tag="ps")
                    nc.tensor.transpose(tp[:], src[:np_, ts(j, P)],
                                        ident[:np_, :np_])
                    nc.scalar.copy(out=dst[:, j, :], in_=tp[:])

            rh64T = pers.tile([P, 3, TS], dt.bfloat16, tag="rh64T")
            pe_transpose(rh64T, rh64[:], TS)

            # ---- kT / qT [128, 4, 512] (padded head-major) ----
            def proj_T(wmat, rhs_src, n, name, bias_name):
                out_sb = pers.tile([P, 4, n], dt.bfloat16, tag=name)
                for m in range(4):
                    ps = psp.tile([P, n], dt.float32, tag="ps")
                    if bias_name in sb_bias:
                        nc.tensor.matmul(ps[:], sb_bias[bias_name][:, ts(m, P)],
                                         ones[:, :n], start=True, stop=False)
                    for k in range(3):
                        nc.tensor.matmul(
                            ps[:], wmat[:, k, ts(m, P)], rhs_src[:, k, :],
                            start=(k == 0 and bias_name not in sb_bias),
                            stop=(k == 2))
                    nc.scalar.copy(out=out_sb[:, m, :], in_=ps[:])
                return out_sb

            kT = proj_T(wk, xhT, RS, "kT", "kb")
            qT = proj_T(wq, rhT, T, "qT", "qb")

            # ---- v row-major [128, 4, 8, 49] with ones column ----
            v_sb = pers.tile([P, 4, NH, KD + 1], dt.bfloat16, tag="v_sb")
            for m in range(4):
                ps = psp.tile([P, C], dt.float32, tag="ps")
                if "vb" in sb_bias:
                    nc.tensor.matmul(ps[:], ones[:, ts(m, P)], sb_bias["vb"][:],
                                     start=True, stop=False)
                for k in range(3):
                    nc.tensor.matmul(ps[:], xhT[:, k, ts(m, P)], wv[:, k, :],
                                     start=(k == 0 and "vb" not in sb_bias),
                                     stop=(k == 2))
                nc.scalar.copy(
                    out=v_sb[:, m, :, 0:KD],
                    in_=ps[:].rearrange("p (h c) -> p h c", h=NH))
                nc.vector.memset(v_sb[:, m, :, KD:KD + 1], 1.0)

            # ---- gate (64-row T shard, 64-padded head stride) ----
            gate64 = pers.tile([TS, HCP], dt.bfloat16, tag="gate64")
            gps = psp.tile([TS, HCP], dt.float32, tag="ps")
            nc.tensor.matmul(gps[:], ones[:, :TS], gb[:], start=True, stop=False)
            for k in range(3):
                nc.tensor.matmul(gps[:], rh64T[:, k, :], wg[:, k, :],
                                 start=False, stop=(k == 2))
            nc.scalar.activation(out=gate64[:], in_=gps[:], func=AF.Sigmoid)

            # ---- attention over heads ----
            NW = NH * (KD + 1)      # 392
            av_ps = [avp.tile([P, NW], dt.float32, tag="av",
                              name="av_ps%d" % i) for i in range(4)]
            for h in range(NH):
                mt, off = h // 2, KDP * (h % 2)
                mask_sb = maskp.tile([P, 4, T], dt.bfloat16, tag="mask")
                nc.gpsimd.dma_start(
                    out=mask_sb[:],
                    in_=d_maskT.ap()[h].rearrange("(m p) t -> p m t", p=P))
                em = emp.tile([P, 4, T], dt.bfloat16, tag="em")
                for rm in range(4):
                    lg = psp.tile([P, T], dt.float32, tag="ps")
                    nc.tensor.matmul(lg[:], kT[off:off + KDP, mt, ts(rm, P)],
                                     qT[off:off + KDP, mt, :],
                                     start=True, stop=True)
                    e_sb = ep.tile([P, T], dt.bfloat16, tag="e")
                    nc.scalar.activation(out=e_sb[:], in_=lg[:], func=AF.Exp)
                    nc.vector.tensor_mul(em[:, rm, :], e_sb[:],
                                         mask_sb[:, rm, :])
                for tm in range(4):
                    for rk in range(4):
                        nc.tensor.matmul(
                            av_ps[tm][:, h * (KD + 1):(h + 1) * (KD + 1)],
                            em[:, rk, ts(tm, P)], v_sb[:, rk, h, :],
                            start=(rk == 0), stop=(rk == 3))

            # ---- partial AV -> dram, ReduceScatter over T ----
            av_in = dram.tile([T, NW], dt.float32, tag="avin")
            av_out = dram.tile([TS, NW], dt.float32, tag="avout")
            av_sb = pers.tile([P, 4, NW], dt.float32, tag="av_sb")
            for tm in range(4):
                nc.vector.tensor_copy(out=av_sb[:, tm, :], in_=av_ps[tm][:])
            nc.gpsimd.dma_start(
                out=av_in[:].rearrange("(m p) n -> p m n", p=P), in_=av_sb[:])
            nc.gpsimd.collective_compute(
                "ReduceScatter", mybir.AluOpType.add,
                replica_groups=[list(range(NCORE))],
                ins=[av_in[:].opt()], outs=[av_out[:].opt()])
            av64 = pers.tile([TS, NW], dt.float32, tag="av64")
            nc.gpsimd.dma_start(out=av64[:], in_=av_out[:])

            # ---- original-side transition (overlaps RS) ----
            t1ot = pers.tile([P, 12, RS], dt.bfloat16, tag="t1ot")
            for fm in range(12):
                ps = psp.tile([P, RS], dt.float32, tag="ps")
                if "b1ot" in sb_bias:
                    nc.tensor.matmul(ps[:], sb_bias["b1ot"][:, ts(fm, P)],
                                     ones[:, :RS], start=True, stop=False)
                for k in range(3):
                    nc.tensor.matmul(
                        ps[:], w1ot[:, k, ts(fm, P)], xhT[:, k, :],
                        start=(k == 0 and "b1ot" not in sb_bias), stop=(k == 2))
                nc.scalar.activation(out=t1ot[:, fm, :], in_=ps[:], func=AF.Relu)
            for m in range(4):
                ps = psp.tile([P, C], dt.float32, tag="ps")
                if "b2ot" in sb_bias:
                    nc.tensor.matmul(ps[:], ones[:, ts(m, P)], sb_bias["b2ot"][:],
                                     start=True, stop=False)
                for f in range(12):
                    nc.tensor.matmul(ps[:], t1ot[:, f, ts(m, P)], w2ot[:, f, :],
                                     start=(f == 0 and "b2ot" not in sb_bias),
                                     stop=(f == 11))
                o_sb = origp.tile([P, C], dt.float32, tag="o_sb")
                nc.vector.tensor_add(out=o_sb[:], in0=x_sb[:, m, :], in1=ps[:])
                nc.sync.dma_start(
                    out=d_orig_out.ap().rearrange("(m p) c -> p m c", p=P)[:, m, :],
                    in_=o_sb[:])

            # ---- post-RS: recip, gate, outproj ----
            av64v = av64[:].rearrange("p (h c) -> p h c", h=NH)
            w64 = pers.tile([TS, HCP], dt.bfloat16, tag="w64")
            w64v = w64[:].rearrange("p (h c) -> p h c", h=NH)
            nc.vector.memset(w64v[:, :, KD:KDP], 0.0)
            recip = pers.tile([TS, NH], dt.float32, tag="recip")
            nc.vector.reciprocal(out=recip[:], in_=av64v[:, :, KD])
            for h in range(NH):
                nc.vector.tensor_scalar_mul(
                    out=w64v[:, h, 0:KD], in0=av64v[:, h, 0:KD],
                    scalar1=recip[:, h:h + 1])
            nc.vector.tensor_mul(w64[:], w64[:], gate64[:])
            wgT = pers.tile([P, 4, TS], dt.bfloat16, tag="wgT")
            at_ps = psp.tile([TS, C], dt.float32, tag="ps")
            if "ob" in sb_bias:
                nc.tensor.matmul(at_ps[:], ones[:, :TS], sb_bias["ob"][:],
                                 start=True, stop=False)
            for k in range(4):
                tp = psp.tile([P, TS], dt.bfloat16, tag="ps", name="tp%d" % k)
                nc.tensor.transpose(tp[:], w64[:, ts(k, P)], ident[:TS, :TS])
                nc.scalar.copy(out=wgT[:, k, :], in_=tp[:])
                nc.tensor.matmul(at_ps[:], wgT[:, k, :], wo[:, k, :],
                                 start=(k == 0 and "ob" not in sb_bias),
                                 stop=(k == 3))
            r2 = pers.tile([TS, C], dt.float32, tag="r2")
            nc.vector.tensor_add(out=r2[:], in0=res64[:], in1=at_ps[:])

            r2h = pers.tile([TS, C], dt.bfloat16, tag="r2h")
            ln(r2h[:], r2[:], TS, "r2")
            r2hT = pers.tile([P, 3, TS], dt.bfloat16, tag="r2hT")
            pe_transpose(r2hT, r2h[:], TS)

            t1rt = pers.tile([P, 12, TS], dt.bfloat16, tag="t1rt")
            for fm in range(12):
                ps = psp.tile([P, TS], dt.float32, tag="ps")
                if "b1rt" in sb_bias:
                    nc.tensor.matmul(ps[:], sb_bias["b1rt"][:, ts(fm, P)],
                                     ones[:, :TS], start=True, stop=False)
                for k in range(3):
                    nc.tensor.matmul(
                        ps[:], w1rt[:, k, ts(fm, P)], r2hT[:, k, :],
                        start=(k == 0 and "b1rt" not in sb_bias), stop=(k == 2))
                nc.vector.tensor_scalar_max(out=t1rt[:, fm, :], in0=ps[:],
                                            scalar1=0.0)
            t2a_ps = psp.tile([TS, C], dt.float32, tag="ps", name="t2a_ps")
            t2b_ps = psp.tile([TS, C], dt.float32, tag="ps", name="t2b_ps")
            if "b2rt" in sb_bias:
                nc.tensor.matmul(t2a_ps[:], ones[:, :TS], sb_bias["b2rt"][:],
                                 start=True, stop=False)
            for f in range(12):
                tgt = t2a_ps if f < 6 else t2b_ps
                nc.tensor.matmul(tgt[:], t1rt[:, f, :], w2rt[:, f, :],
                                 start=(f % 6 == 0
                                        and not (f == 0 and "b2rt" in sb_bias)),
                                 stop=(f % 6 == 5))
            res_fin = pers.tile([TS, C], dt.float32, tag="res_fin")
            nc.vector.tensor_add(out=res_fin[:], in0=r2[:], in1=t2a_ps[:])
            nc.vector.tensor_add(out=res_fin[:], in0=res_fin[:], in1=t2b_ps[:])
            nc.sync.dma_start(out=d_res_out.ap(), in_=res_fin[:])

    nc.compile()
    return nc


def kernel(original, resampled, attention_mask, output_mask, input_mask,
           qn_scale, qn_offset, dn_scale, dn_offset,
           query_w, key_w, value_w, gating_w, gating_b, output_w, output_b,
           rt_ln_scale, rt_ln_offset, rt_w1, rt_b1, rt_w2, rt_b2,
           ot_ln_scale, ot_ln_offset, ot_w1, ot_b1, ot_w2, ot_b2):
    from concourse.bass_utils import run_bass_kernel_spmd

    f32 = np.float32
    original = np.asarray(original, f32)
    resampled = np.asarray(resampled, f32)
    attention_mask = np.asarray(attention_mask, f32)

    def pad_qk(w):  # [C, H, KD] -> [C, HCP]
        wp = np.zeros((C, NH, KDP), f32)
        wp[:, :, :KD] = w
        return wp.reshape(C, HCP)

    qw = np.asarray(query_w, f32) * (KD ** -0.5)
    wq_eff = pad_qk(np.asarray(qn_scale, f32)[:, None, None] * qw)
    qb = np.einsum("c,chv->hv", np.asarray(qn_offset, f32), qw)
    qb_p = np.zeros((NH, KDP), f32)
    qb_p[:, :KD] = qb
    wk_eff = pad_qk(np.asarray(dn_scale, f32)[:, None, None]
                    * np.asarray(key_w, f32))
    kb = np.einsum("c,chv->hv", np.asarray(dn_offset, f32),
                   np.asarray(key_w, f32))
    kb_p = np.zeros((NH, KDP), f32)
    kb_p[:, :KD] = kb
    wv_eff = (np.asarray(dn_scale, f32)[:, None, None]
              * np.asarray(value_w, f32)).reshape(C, C)
    vb = np.einsum("c,chv->hv", np.asarray(dn_offset, f32),
                   np.asarray(value_w, f32)).reshape(C)
    wg_eff = pad_qk(np.asarray(qn_scale, f32)[:, None, None]
                    * np.asarray(gating_w, f32))
    gb_hv = (np.asarray(gating_b, f32)
             + np.einsum("c,chv->hv", np.asarray(qn_offset, f32),
                         np.asarray(gating_w, f32)))
    gb = np.zeros((NH, KDP), f32)
    gb[:, :KD] = gb_hv
    gb = gb.reshape(HCP)
    wo_eff = np.zeros((NH, KDP, C), f32)
    wo_eff[:, :KD, :] = np.asarray(output_w, f32)
    wo_eff = wo_eff.reshape(HCP, C)
    ob = np.asarray(output_b, f32)
    w1ot_eff = np.asarray(ot_ln_scale, f32)[:, None] * np.asarray(ot_w1, f32)
    b1ot = np.asarray(ot_b1, f32) + np.asarray(ot_ln_offset, f32) @ np.asarray(ot_w1, f32)
    b2ot = np.asarray(ot_b2, f32)
    w1rt_eff = np.asarray(rt_ln_scale, f32)[:, None] * np.asarray(rt_w1, f32)
    b1rt = np.asarray(rt_b1, f32) + np.asarray(rt_ln_offset, f32) @ np.asarray(rt_w1, f32)
    b2rt = np.asarray(rt_b2, f32)

    bias_vals = {"qb": qb_p.reshape(1, HCP), "kb": kb_p.reshape(1, HCP),
                 "vb": vb.reshape(1, C), "ob": ob.reshape(1, C),
                 "b1ot": b1ot.reshape(1, F), "b2ot": b2ot.reshape(1, C),
                 "b1rt": b1rt.reshape(1, F), "b2rt": b2rt.reshape(1, C)}
    biases = {k: bool(np.any(v != 0)) for k, v in bias_vals.items()}

    key = tuple(sorted(biases.items()))
    if key not in _CACHE:
        _CACHE[key] = _build_program(biases)
    nc = _CACHE[key]

    shared = {
        "res": resampled[0],
        "wq": wq_eff.astype(BF16), "wk": wk_eff.astype(BF16),
        "wv": wv_eff.astype(BF16), "wg": wg_eff.astype(BF16),
        "wo": wo_eff.astype(BF16),
        "gb": gb.reshape(1, HCP).astype(BF16),
        "w1ot": w1ot_eff.astype(BF16), "w2ot": np.asarray(ot_w2, f32).astype(BF16),
        "w1rt": w1rt_eff.astype(BF16), "w2rt": np.asarray(rt_w2, f32).astype(BF16),
    }
    for name, on in biases.items():
        if on:
            shared[name] = bias_vals[name].astype(BF16)

    in_maps = []
    for i in range(NCORE):
        m = dict(shared)
        m["x"] = np.ascontiguousarray(original[0, RS * i:RS * (i + 1)])
        m["res64"] = np.ascontiguousarray(resampled[0, TS * i:TS * (i + 1)])
        m["maskT"] = np.ascontiguousarray(
            attention_mask[:, :, RS * i:RS * (i + 1)].transpose(0, 2, 1)
        ).astype(BF16)
        in_maps.append(m)

    trace = os.environ.get("BASS_KERNEL_TRACE", "") == "1"
    br = run_bass_kernel_spmd(nc, in_maps, list(range(NCORE)), trace=trace)
    LAST["exec_time_ns"] = br.exec_time_ns
    LAST["results"] = br

    res_full = np.concatenate([br.results[i]["res_out"] for i in range(NCORE)],
                              axis=0)[None]
    orig_full = np.concatenate([br.results[i]["orig_out"] for i in range(NCORE)],
                               axis=0)[None]
    return res_full.astype(f32), orig_full.astype(f32)
